# revision 8
# baseline (speedup 1.0000x reference)
"""Trainium2 Bass kernel for nn_HAN_Integrated (GatedGraph message passing).

Math per iteration (reference.py):
    act[e]  = edge_matrix[e].T @ h + ba            # [N,S] per edge type
    z       = sigmoid(sum_e act[e] @ wz[e] + h @ uz)
    r       = sigmoid(sum_e act[e] @ wr[e] + h @ ur)
    hh      = tanh  (sum_e act[e] @ wh[e] + (r*h) @ uh)
    h       = (1-z)*h + r*hh                        # 10 iterations

Sharding: columns (n) of the [E,N,N] adjacency are row-partitioned across
8 cores (NL=375 nodes per core). Each core computes act/z/r/hh/h_new for
its 375 nodes; an AllGather rebuilds the full [N,S] h each iteration.

On-chip compute uses a TRANSPOSED [S, n_local] layout so the streamed edge
matrix is always the PE's *moving* operand.

Precision: the recurrence is chaotic (~300x l2 error amplification over the
10 iterations), so per-iteration compute noise must stay at fp32-like
levels. Version "v2" runs the dominant matmuls in fp16 *pairs*
(Dekker-style hi/lo splits, ~22-bit effective mantissa) at full PE rate:
  pass A: rhs = em_hi(fp16),        lhsT = [h_hi | h_lo] (M=128 stack)
  pass B: rhs = em_lo*2048 (fp16),  lhsT = h_hi * 2^-11
both accumulating into one PSUM group, so act = psum_top + psum_bot.
Stage B runs [w_hi | w_lo] M=128 stacks against act_hi/act_lo. The ba bias
is folded into the gate activations (host-precomputed ba @ sum_e w[e]).
Version "f32" is the exact-fp32 fallback (4x slower PE).
"""

import sys

for _p in ("/opt/trn_rl_repo", "/opt/pypackages"):
    if _p not in sys.path:
        sys.path.insert(0, _p)

import numpy as np

import concourse.bacc as bacc
import concourse.mybir as mybir
from concourse import masks
from concourse.bass_utils import run_bass_kernel_spmd
from concourse.tile import TileContext

N, S, E = 3000, 64, 12
NCORES = 8
NL = N // NCORES          # 375 nodes per core
MT = 125                  # contraction (m) tile -> partition dim
T = N // MT               # 24 m-tiles
ITERS = 10
F32 = mybir.dt.float32
F16 = mybir.dt.float16
AF = mybir.ActivationFunctionType
LO_SCALE = 2048.0         # em_lo stored as fp16 * 2048 (keeps values normal)

VERSION = "v2"            # "v2" fp16-pair (fast) | "f32" exact fp32
EM_BUFS = 3


def _build_v2(nc, tc, pools, tensors, iters):
    (cpool, empool_h, empool_l, hpool, h2pool, htpool, apool, gpool,
     papool, pgpool, trpool, dpool, dspool) = pools
    (emh_d, eml_d, h0_d, h0T_d, wst_d, ust_d, bias_d, out_d) = tensors
    rg = [list(range(NCORES))]

    ident = cpool.tile([128, 128], F32, name="ident")
    masks.make_identity(nc, ident[:])

    wst = {}
    for nm in ("wz", "wr", "wh"):
        t = cpool.tile([S, E * 2 * S], F16, name=f"{nm}_sb")
        nc.sync.dma_start(out=t[:], in_=wst_d[nm][:])
        wst[nm] = t
    ust = {}
    for nm in ("uz", "ur", "uh"):
        t = cpool.tile([S, 2 * S], F16, name=f"{nm}_sb")
        nc.sync.dma_start(out=t[:], in_=ust_d[nm][:])
        ust[nm] = t
    bias_sb = {}
    for nm in ("bzn", "br", "bh"):
        t = cpool.tile([S, 1], F32, name=f"{nm}_sb")
        nc.sync.dma_start(out=t[:], in_=bias_d[nm][:])
        bias_sb[nm] = t

    hT_cur = htpool.tile([S, NL], F32, tag="hT")
    nc.scalar.dma_start(out=hT_cur[:], in_=h0T_d[:])

    ag_out = None
    for it in range(iters):
        h_cur = hpool.tile([MT, T * S], F32, tag="h")
        src = h0_d[:] if it == 0 else ag_out[:]
        nc.scalar.dma_start(
            out=h_cur[:].rearrange("p (t s) -> p t s", t=T),
            in_=src.rearrange("(t p) s -> p t s", p=MT),
        )
        # fp16 hi/lo splits of h: h2 = [hi | lo] per m-tile, h2b = hi * 2^-11
        h2 = h2pool.tile([MT, T * 2 * S], F16, tag="h2")
        h2b = h2pool.tile([MT, T * S], F16, tag="h2b")
        hhf = h2pool.tile([MT, T * S], F32, tag="hhf")
        hc3 = h_cur[:].rearrange("p (t s) -> p t s", t=T)
        h2v = h2[:].rearrange("p (t j s) -> p t j s", t=T, j=2)
        nc.vector.tensor_copy(h2v[:, :, 0, :], hc3)           # h_hi (fp16)
        nc.vector.tensor_copy(hhf[:].rearrange("p (t s) -> p t s", t=T), h2v[:, :, 0, :])
        nc.vector.tensor_sub(h2v[:, :, 1, :], hc3,
                             hhf[:].rearrange("p (t s) -> p t s", t=T))  # h_lo
        nc.vector.tensor_scalar_mul(
            h2b[:].rearrange("p (t s) -> p t s", t=T),
            h2v[:, :, 0, :], 1.0 / LO_SCALE)

        pz = pgpool.tile([2 * S, NL], F32, tag="pz")
        pr = pgpool.tile([2 * S, NL], F32, tag="pr")
        ph = pgpool.tile([2 * S, NL], F32, tag="ph")

        for e in range(E):
            emh_t = empool_h.tile([MT, T * NL], F16, tag="emh")
            nc.sync.dma_start(out=emh_t[:], in_=emh_d[e])
            eml_t = empool_l.tile([MT, T * NL], F16, tag="eml")
            nc.sync.dma_start(out=eml_t[:], in_=eml_d[e])
            pa = papool.tile([2 * S, NL], F32, tag="pa")
            for t in range(T):
                nc.tensor.matmul(
                    pa[:], lhsT=h2[:, t * 2 * S:(t + 1) * 2 * S],
                    rhs=emh_t[:, t * NL:(t + 1) * NL],
                    start=(t == 0), stop=False, skip_group_check=True)
            for t in range(T):
                nc.tensor.matmul(
                    pa[0:S, :], lhsT=h2b[:, t * S:(t + 1) * S],
                    rhs=eml_t[:, t * NL:(t + 1) * NL],
                    start=False, stop=(t == T - 1), skip_group_check=True)

            pa_bot = apool.tile([S, NL], F32, tag="pabot")
            nc.scalar.copy(pa_bot[:], pa[S:2 * S, :])
            act_f = apool.tile([S, NL], F32, tag="actf")
            nc.vector.tensor_add(act_f[:], pa[0:S, :], pa_bot[:])
            a_hi = apool.tile([S, NL], F16, tag="ahi")
            nc.vector.tensor_copy(a_hi[:], act_f[:])
            a_hif = apool.tile([S, NL], F32, tag="ahif")
            nc.vector.tensor_copy(a_hif[:], a_hi[:])
            a_lo = apool.tile([S, NL], F16, tag="alo")
            nc.vector.tensor_sub(a_lo[:], act_f[:], a_hif[:])
            for nm, pg in (("wz", pz), ("wr", pr), ("wh", ph)):
                for rhs in (a_hi, a_lo):
                    nc.tensor.matmul(
                        pg[:], lhsT=wst[nm][:, e * 2 * S:(e + 1) * 2 * S],
                        rhs=rhs[:], start=(e == 0 and rhs is a_hi),
                        stop=False, skip_group_check=True)

        # u-terms: hT hi/lo against [u_hi | u_lo] stacks
        hT_hi = gpool.tile([S, NL], F16, tag="hthi")
        nc.vector.tensor_copy(hT_hi[:], hT_cur[:])
        hT_hif = gpool.tile([S, NL], F32, tag="hthif")
        nc.vector.tensor_copy(hT_hif[:], hT_hi[:])
        hT_lo = gpool.tile([S, NL], F16, tag="htlo")
        nc.vector.tensor_sub(hT_lo[:], hT_cur[:], hT_hif[:])
        for u_nm, pg, last in (("uz", pz, True), ("ur", pr, True)):
            nc.tensor.matmul(pg[:], lhsT=ust[u_nm][:], rhs=hT_hi[:],
                             start=False, stop=False, skip_group_check=True)
            nc.tensor.matmul(pg[:], lhsT=ust[u_nm][:], rhs=hT_lo[:],
                             start=False, stop=last, skip_group_check=True)

        gzb = gpool.tile([S, NL], F32, tag="gzb")
        nc.scalar.copy(gzb[:], pz[S:2 * S, :])
        gz = gpool.tile([S, NL], F32, tag="gz")
        nc.vector.tensor_add(gz[:], pz[0:S, :], gzb[:])
        omz = gpool.tile([S, NL], F32, tag="omz")
        nc.scalar.activation(omz[:], gz[:], AF.Sigmoid, scale=-1.0,
                             bias=bias_sb["bzn"][:, 0:1])
        grb = gpool.tile([S, NL], F32, tag="grb")
        nc.scalar.copy(grb[:], pr[S:2 * S, :])
        gr = gpool.tile([S, NL], F32, tag="gr")
        nc.vector.tensor_add(gr[:], pr[0:S, :], grb[:])
        r_sb = gpool.tile([S, NL], F32, tag="r")
        nc.scalar.activation(r_sb[:], gr[:], AF.Sigmoid,
                             bias=bias_sb["br"][:, 0:1])

        rh = gpool.tile([S, NL], F32, tag="rh")
        nc.vector.tensor_mul(rh[:], r_sb[:], hT_cur[:])
        rh_hi = gpool.tile([S, NL], F16, tag="rhhi")
        nc.vector.tensor_copy(rh_hi[:], rh[:])
        rh_hif = gpool.tile([S, NL], F32, tag="rhhif")
        nc.vector.tensor_copy(rh_hif[:], rh_hi[:])
        rh_lo = gpool.tile([S, NL], F16, tag="rhlo")
        nc.vector.tensor_sub(rh_lo[:], rh[:], rh_hif[:])
        nc.tensor.matmul(ph[:], lhsT=ust["uh"][:], rhs=rh_hi[:],
                         start=False, stop=False, skip_group_check=True)
        nc.tensor.matmul(ph[:], lhsT=ust["uh"][:], rhs=rh_lo[:],
                         start=False, stop=True, skip_group_check=True)
        ghb = gpool.tile([S, NL], F32, tag="ghb")
        nc.scalar.copy(ghb[:], ph[S:2 * S, :])
        gh = gpool.tile([S, NL], F32, tag="gh")
        nc.vector.tensor_add(gh[:], ph[0:S, :], ghb[:])
        hh = gpool.tile([S, NL], F32, tag="hh")
        nc.scalar.activation(hh[:], gh[:], AF.Tanh, bias=bias_sb["bh"][:, 0:1])

        m1 = gpool.tile([S, NL], F32, tag="m1")
        nc.vector.tensor_mul(m1[:], omz[:], hT_cur[:])
        m2 = gpool.tile([S, NL], F32, tag="m2")
        nc.vector.tensor_mul(m2[:], r_sb[:], hh[:])
        hT_new = htpool.tile([S, NL], F32, tag="hT")
        nc.vector.tensor_add(hT_new[:], m1[:], m2[:])
        hT_cur = hT_new

        hn_sb = gpool.tile([MT, 3 * S], F32, tag="hn")
        if it < iters - 1:
            ag_in = dpool.tile([NL, S], F32, tag="ag_in")
            for c in range(3):
                ptr = trpool.tile([MT, S], F32, tag="ptr")
                nc.tensor.transpose(ptr[:], hT_new[:, c * MT:(c + 1) * MT],
                                    ident[:S, :S])
                nc.scalar.copy(hn_sb[:, c * S:(c + 1) * S], ptr[:])
                nc.scalar.dma_start(out=ag_in[c * MT:(c + 1) * MT, :],
                                    in_=hn_sb[:, c * S:(c + 1) * S])
            ag_out = dspool.tile([N, S], F32, tag="ag_out", addr_space="Shared")
            nc.gpsimd.collective_compute(
                "AllGather", mybir.AluOpType.bypass, replica_groups=rg,
                ins=[ag_in[:].opt()], outs=[ag_out[:].opt()])
        else:
            for c in range(3):
                ptr = trpool.tile([MT, S], F32, tag="ptr")
                nc.tensor.transpose(ptr[:], hT_new[:, c * MT:(c + 1) * MT],
                                    ident[:S, :S])
                nc.scalar.copy(hn_sb[:, c * S:(c + 1) * S], ptr[:])
                nc.scalar.dma_start(out=out_d[c * MT:(c + 1) * MT, :],
                                    in_=hn_sb[:, c * S:(c + 1) * S])


def build_module_v2(em_bufs: int = EM_BUFS, iters: int = ITERS):
    nc = bacc.Bacc("TRN2", target_bir_lowering=False, debug=False,
                   num_devices=NCORES)
    emh_d = nc.dram_tensor("emh", [E, MT, T * NL], F16, kind="ExternalInput")
    eml_d = nc.dram_tensor("eml", [E, MT, T * NL], F16, kind="ExternalInput")
    h0_d = nc.dram_tensor("h0", [N, S], F32, kind="ExternalInput")
    h0T_d = nc.dram_tensor("h0T", [S, NL], F32, kind="ExternalInput")
    wst_d = {nm: nc.dram_tensor(nm, [S, E * 2 * S], F16, kind="ExternalInput")
             for nm in ("wz", "wr", "wh")}
    ust_d = {nm: nc.dram_tensor(nm, [S, 2 * S], F16, kind="ExternalInput")
             for nm in ("uz", "ur", "uh")}
    bias_d = {nm: nc.dram_tensor(nm, [S, 1], F32, kind="ExternalInput")
              for nm in ("bzn", "br", "bh")}
    out_d = nc.dram_tensor("out", [NL, S], F32, kind="ExternalOutput")

    with TileContext(nc) as tc:
        with (
            tc.tile_pool(name="const", bufs=1) as cpool,
            tc.tile_pool(name="empool_h", bufs=em_bufs) as empool_h,
            tc.tile_pool(name="empool_l", bufs=em_bufs) as empool_l,
            tc.tile_pool(name="hpool", bufs=2) as hpool,
            tc.tile_pool(name="h2pool", bufs=1) as h2pool,
            tc.tile_pool(name="htpool", bufs=2) as htpool,
            tc.tile_pool(name="apool", bufs=2) as apool,
            tc.tile_pool(name="gpool", bufs=1) as gpool,
            tc.tile_pool(name="papool", bufs=2, space="PSUM") as papool,
            tc.tile_pool(name="pgpool", bufs=1, space="PSUM") as pgpool,
            tc.tile_pool(name="trpool", bufs=3, space="PSUM") as trpool,
            tc.tile_pool(name="dpool", bufs=2, space="DRAM") as dpool,
            tc.tile_pool(name="dspool", bufs=2, space="DRAM") as dspool,
        ):
            _build_v2(nc, tc,
                      (cpool, empool_h, empool_l, hpool, h2pool, htpool,
                       apool, gpool, papool, pgpool, trpool, dpool, dspool),
                      (emh_d, eml_d, h0_d, h0T_d, wst_d, ust_d, bias_d, out_d),
                      iters)
    nc.finalize()
    return nc


def make_in_maps_v2(x, edge_matrix, ba, wz, wr, wh, uz, ur, uh):
    x = np.ascontiguousarray(np.asarray(x, np.float32))
    em = np.asarray(edge_matrix, np.float32)
    ba = np.asarray(ba, np.float32)

    def pair_stack(w):
        # [E,S,S] -> [S, E*2*S] with per-e block [hi(64) | lo(64)] on free axis
        w = np.asarray(w, np.float32)
        hi = w.astype(np.float16)
        lo = (w - hi.astype(np.float32)).astype(np.float16)
        st = np.stack([hi, lo], axis=2)            # [E, S, 2, S]
        return np.ascontiguousarray(st.transpose(1, 0, 2, 3).reshape(S, E * 2 * S))

    def upair(u):
        u = np.asarray(u, np.float32)
        hi = u.astype(np.float16)
        lo = (u - hi.astype(np.float32)).astype(np.float16)
        return np.ascontiguousarray(np.concatenate([hi, lo], axis=1))  # [S, 2S]

    w_h = {"wz": pair_stack(wz), "wr": pair_stack(wr), "wh": pair_stack(wh)}
    u_h = {"uz": upair(uz), "ur": upair(ur), "uh": upair(uh)}
    bz = (ba.astype(np.float64) @ np.asarray(wz, np.float64).sum(0)).astype(np.float32)
    br = (ba.astype(np.float64) @ np.asarray(wr, np.float64).sum(0)).astype(np.float32)
    bh = (ba.astype(np.float64) @ np.asarray(wh, np.float64).sum(0)).astype(np.float32)
    biases = {"bzn": np.ascontiguousarray(-bz.reshape(S, 1)),
              "br": np.ascontiguousarray(br.reshape(S, 1)),
              "bh": np.ascontiguousarray(bh.reshape(S, 1))}

    in_maps = []
    for rr in range(NCORES):
        n0 = rr * NL
        shard = em[:, :, n0:n0 + NL].reshape(E, T, MT, NL)
        shard = np.ascontiguousarray(shard.transpose(0, 2, 1, 3)).reshape(
            E, MT, T * NL)
        s_hi = shard.astype(np.float16)
        s_lo = ((shard - s_hi.astype(np.float32)) * LO_SCALE).astype(np.float16)
        in_maps.append({
            "emh": s_hi, "eml": s_lo,
            "h0": x,
            "h0T": np.ascontiguousarray(x[n0:n0 + NL].T),
            **w_h, **u_h, **biases,
        })
    return in_maps


_NC_CACHE = {}


def get_nc(version: str = VERSION, em_bufs: int = EM_BUFS, iters: int = ITERS):
    key = (version, em_bufs, iters)
    if key not in _NC_CACHE:
        if version == "v2":
            _NC_CACHE[key] = build_module_v2(em_bufs, iters)
        else:
            raise ValueError(version)
    return _NC_CACHE[key]


def run(inputs, trace=False, version: str = VERSION, em_bufs: int = EM_BUFS,
        iters: int = ITERS):
    in_maps = make_in_maps_v2(
        inputs["x"], inputs["edge_matrix"], inputs["ba"],
        inputs["wz"], inputs["wr"], inputs["wh"],
        inputs["uz"], inputs["ur"], inputs["uh"],
    )
    res = run_bass_kernel_spmd(get_nc(version, em_bufs, iters), in_maps,
                               core_ids=list(range(NCORES)), trace=trace)
    out = np.concatenate([res.results[r]["out"] for r in range(NCORES)], axis=0)
    return np.ascontiguousarray(out, dtype=np.float32), res


def kernel(x, edge_matrix, ba, wz, wr, wh, uz, ur, uh, iteration):
    assert int(iteration) == ITERS, f"kernel hardcodes {ITERS} iterations"
    out, _ = run({"x": x, "edge_matrix": edge_matrix, "ba": ba,
                  "wz": wz, "wr": wr, "wh": wh,
                  "uz": uz, "ur": ur, "uh": uh})
    return out
